# revision 38
# baseline (speedup 1.0000x reference)
"""Multi-head attention kernel for Trainium2, SPMD across 8 NeuronCores.

Problem: b=2, s=2048, d_model=1024, 16 heads x 64 dims, packed QKV proj,
softmax over keys (boolean key mask), out-projection.

Sharding: core c in 0..7 handles batch b = c//4 and a group of 4 heads
g = c%4 (data parallel over batch x head/tensor parallel).  Each core
computes its head-group's out-projection partial [2048, 1024]; the host
sums the 4 partials per batch (the row-parallel reduction) and upcasts
from bf16.

Architecture: a FLATTENED 4-loop software pipeline (loop li = pair*2 +
sq-half), one continuous St/exp stream across all loop boundaries.

PSUM (8 banks, one pool, three independently-rotating tags):
  st   2x [128,1024] = 4 banks: score tiles ONLY.  The rotation is paced
       purely by the exp drain, so injected work can never displace a
       score tile (the baseline's single shared rotation cost ~50us of
       exp-stream stalls).
  accA 2x [65,512] halves = 2 banks: head-A PV accumulator, resident a
       whole loop.
  inj  2x [128,512] = 2 banks: serial injection stream (V-proj, Q/K
       quarter-projections, out-proj halves, PE-broadcast tiles) and the
       head-B PV accumulator halves, claimed at k=SB[li] as the LAST inj
       allocs of the loop.

Split-head PV: head-A PV trails St by 2; head-B PV starts at SB[li]
(~12) reading pt tiles buffered in SBUF (ptA tag 6 bufs, ptB 16), and
its remainder drains 3-per-iteration into the NEXT loop's first k's,
followed by the accumulator evictions and the normalize chain -- the
exp stream never pauses at a j-boundary.

exp: ScalarE activation Exp (scale=1/8, ~1.11us per [128,1024] tile) +
DVE co-exps (custom 2-op cubic^64, 2.44us/pair) on head-B tiles per
DVE_EXP_LI.  Placement is the key lever: in the PE-thin k=12..15 tail
of each loop the pipeline is ScalarE-paced at ~77% PE duty, which trips
the HAM half-clock gate; a DVE co-exp every other k there makes the
stretch PE-paced at full duty (no idle window, no gate), and co-exps on
the quarter-projection k's absorb their PE load.  Co-exps must stay
clear of k<=8 where the previous loop's normalize chain occupies the
in-order DVE queue.

normalize (mid-loop): acc halves evict to SBUF (DVE), rowsum rows
DMA-hop to one [1,2048] tile, per-head reciprocals, then gpsimd
partition_broadcast + DVE multiplies.  The broadcast MUST be gpsimd
here: a PE-matmul broadcast sits behind ~12 k-iterations of queued
matmuls and the DVE multiplies waiting on it fill the DVE's 4-deep
wait queue, blocking the co-exps (measured 8.5us/boundary).  gpsimd
runs ONLY partition_broadcast + input SWDGE DMAs (no ucode thrash).
At the TAIL the broadcast is tiny PE matmuls instead (PSUM free, PE
idle); the rowsum hops stay on sync HWDGE (gpsimd SWDGE descriptor
generation costs ~1.3us/DMA and delayed the recip by ~3.5us), and a
3-matmul discarded bridge after slice 6 covers the recipB latency so
the PE never idles into the half-clock gate.

out-proj: slices 0-4 injected into loop 3 (k=8..12, inj tag); slices
5-15 in the tail on the freed st+accA tags (4-deep slot pipeline;
evictions split DVE/ScalarE, output DMAs striped sync/scalar).  Slices
5-7 need only the j0 ot tiles so their matmuls bridge the head-B
normalize chain.

Input DMA: issues striped sync/scalar in exact consumption order
(pair-0 wq | wk halves, xt-half0 alternating, xt-half1, pair-1 weight
halves), non-critical wv/mask/wo on
the gpsimd SWDGE queue.  Lead-in projects ONLY pair-0 Q/K half-0,
Q and K interleaved per c-chunk so each chunk's matmuls fire as its
DMA lands; half-1 and pair-1 projections are injected into the loops
(quarters, deadlines per consumption).  A dummy 1-element exp pulls
the ACT-table load into the DMA lead-in.

Measured: 217.6us at full clock, HAM warm end-to-end (baseline 253.5us; throttled
runs read ~258us).  Without the loop-3 k=5..7 ST_REPS filler the HAM
gate stochastically costs a 34us half-clock window there (233us draws);
the 12-matmul filler pins it warm at ~1.5% extra PE power.  Known HW traps:
discarded-matmul filler at scale trips a DEVICE power throttle to
2.0GHz (everything stretches exactly 1.2x -- that throttle also
appears stochastically on some runs regardless); gpsimd cannot read
PSUM; scalar.dma_start blocks the exp stream; 1-lane DVE ops based at
partition 64 return garbage; fp32/bf16 matmul operand mixing is
rejected; an inj-tag alloc emitted after the accB claim of the same
loop head-of-line-blocks the tensor queue until the next loop's drain.
"""

import numpy as np
import ml_dtypes

BF = ml_dtypes.bfloat16
S = 2048
C = 1024
DQ = 64
HL = 4  # local heads per core
KT = S // 128  # 16 key tiles
CT = C // 128  # 8 contraction tiles
SCALE = 8.0  # sqrt(DQ)

# exp(s/8) = p3(s/512)^64, p3 = cubic Taylor of e^v at 0
_R = 1.0 / 512.0
EXPC_S0 = _R * _R * _R / 6.0  # v^3 coeff (on raw scores)
EXPC_S1 = _R * _R / 2.0  # v^2 coeff
EXPC_IMM2 = _R  # v^1 coeff
# (k, head) pairs whose exp runs on DVE, per (pair, j).  ONE head per k
# (the other head's exp stays on ScalarE in parallel), kept away from the
# first/last few k so the j-boundary normalize chain and the psA ping-pong
# restart never queue behind slow DVE exps (2 x 1.22us each).  Only the
# loops without injected PE filler get DVE exps -- the padded loops are
# PE-bound and ScalarE alone keeps up.
# pair-1 j0 also carries the double-St density guard: 8 DVE exps there
# compound with the doubled score matmuls into a 2.9us/k crawl -- 4 is the
# sweet spot (measured).
DVE_EXP_KI = {
    (1, 0): {(5, 1), (7, 0), (9, 1), (11, 0)},
    (1, 1): {(5, 1), (7, 0), (9, 1), (11, 0)},
}

_CACHED = None
_DVE_OPS = None


def _register_dve_ops():
    """Register the two custom DVE exp uOps into concourse's per-process op
    table (the repo is read-only, so dve_ops.py can't be edited; appending
    to the module-level registry at runtime is equivalent -- the per-NEFF
    DVE table generator and the ISA row lookup both read these dicts)."""
    global _DVE_OPS
    if _DVE_OPS is not None:
        return _DVE_OPS
    import concourse.dve_ops as dve_ops
    from concourse.dve_ops import DveOp
    from concourse.dve_spec import Spec, Src0, C0, C1, C2, One, sq, lower
    from concourse.dve_uop import DveOpSpec

    def ref_cubic(in0, in1, s0, s1, imm2):
        x = in0.astype(np.float32)
        return (
            (np.float32(s0) * x + np.float32(s1)) * x + np.float32(imm2)
        ) * x + np.float32(1.0)

    def ref_pow64(in0, in1, s0, s1, imm2):
        x = in0.astype(np.float32)
        for _ in range(6):
            x = (x * x).astype(np.float32)
        return x

    body1 = ((C0 * Src0 + C1) * Src0 + C2) * Src0 + One
    b = Src0
    for _ in range(6):
        b = sq(b)
    specs = [
        ("EXP_CUBIC_ANTK", Spec(body=body1, reference=ref_cubic)),
        ("POW64_ANTK", Spec(body=b, reference=ref_pow64)),
    ]
    ops = []
    for name, spec in specs:
        if name in dve_ops._SUB_OPCODE_FOR_NAME:
            ops.append(next(o for o in dve_ops.OPS if o.name == name))
            continue
        row = dve_ops._CUSTOM_DVE_ROW_BASE + len(dve_ops.OPS)
        assert row < 0x20
        shas = {}
        for ver in ("v3", "v4"):
            s = DveOpSpec(
                name=name, opcode=row, uops=lower(spec, ver=ver), rd1_en=False
            )
            shas[ver] = s.sha(ver)
        op = DveOp(name, spec, subdim=False, uops_sha=shas)
        dve_ops.OPS.append(op)
        dve_ops._SUB_OPCODE_FOR_NAME[name] = row
        dve_ops.CUSTOM_DVE_SPECS[name] = spec
        ops.append(op)
    _DVE_OPS = tuple(ops)
    return _DVE_OPS


def _build():
    import concourse.bacc as bacc
    import concourse.mybir as mybir
    import concourse.tile as tile

    EXP_CUBIC, POW64 = _register_dve_ops()

    F32 = mybir.dt.float32
    BF16 = mybir.dt.bfloat16
    EXP = mybir.ActivationFunctionType.Exp

    nc = bacc.Bacc(
        "TRN2",
        target_bir_lowering=False,
        debug=False,
        enable_asserts=False,
        num_devices=8,
    )

    XT = nc.dram_tensor("xt", [C, S], BF16, kind="ExternalInput").ap()
    WQ = nc.dram_tensor("wq", [128, CT * 256], BF16, kind="ExternalInput").ap()
    WK = nc.dram_tensor("wk", [128, CT * 256], BF16, kind="ExternalInput").ap()
    WV = nc.dram_tensor("wv", [C, 2 * 128], BF16, kind="ExternalInput").ap()
    WO = nc.dram_tensor("wo", [HL * DQ, C], BF16, kind="ExternalInput").ap()
    MV = nc.dram_tensor("maskv", [128, KT], F32, kind="ExternalInput").ap()
    OUT = nc.dram_tensor("out", [S, C], BF16, kind="ExternalOutput").ap()

    from contextlib import ExitStack
    from collections import defaultdict

    with tile.TileContext(nc) as tc:
        with ExitStack() as stack:
            pool = lambda *a, **k: stack.enter_context(tc.tile_pool(*a, **k))
            p_xt = pool(name="xt", bufs=CT)
            p_w = pool(name="wqk", bufs=2)
            p_wv = pool(name="wv", bufs=CT)
            p_wo = pool(name="wo", bufs=2)
            p_c = pool(name="cst", bufs=1)
            p_qk = pool(name="qk", bufs=4)
            p_v = pool(name="v", bufs=KT)
            p_pt = pool(name="pt", bufs=2)
            p_h = pool(name="h", bufs=2)
            p_ev = pool(name="ev", bufs=2)
            p_rc = pool(name="rc", bufs=2)
            p_rs = pool(name="rs", bufs=2)
            p_ot = pool(name="ot", bufs=4)
            p_bc = pool(name="bc", bufs=1)
            p_os = pool(name="os", bufs=8)
            # ONE psum pool, tag-partitioned (tags rotate independently):
            #   st   2x [128,1024] f32 = 4 banks -- score tiles ONLY; the
            #        rotation is paced purely by the exp drain, so the exp
            #        stream can never be displaced by injected work
            #   accA 2x [65,512]   f32 = 2 banks -- head-A PV accumulator
            #        (n-halves), resident for a whole j-loop
            #   inj  2x [128,512]  f32 = 2 banks -- serial injection stream:
            #        V-proj / Q,K quarter-projs / out-proj halves / bcast
            #        tiles, and the head-B PV accumulator halves at loop end
            ps = pool(name="ps", bufs=2, space="PSUM")
            # ---------------- input DMA ----------------
            # The DMA ISSUE cost is ~600ns per instruction on a queue
            # regardless of size (cost ~ partition-row count), and there are
            # two HWDGE queues (sync + scalar).  The baseline put 43 issues
            # on sync alone = ~26us of serialized issue -- the first exp
            # waited 29us on it.  Stripe the issues across BOTH queues in
            # exact consumption order (Q-half0 needs wq + xt-half0, K-half0
            # adds wk, V needs wv + xt-half0 cols, half1 traffic last), and
            # merge each weight into ONE whole-tile DMA.
            wq_sb = p_w.tile([128, CT * 256], BF16, tag="wq", name="wq_sb")
            wk_sb = p_w.tile([128, CT * 256], BF16, tag="wk", name="wk_sb")
            xt_t = [p_xt.tile([128, S], BF16, tag="xt", name="xt_t") for _ in range(CT)]
            wv_t = []
            for c in range(CT):
                wv_t.append(p_wv.tile([128, HL * DQ], BF16, tag="wv", name="wv_t"))
            # critical stream (wq + wk + xt-half0 + xt-half1) striped across
            # the two HWDGE queues; ALL non-critical input (wv, mask, wo)
            # goes through the otherwise-idle gpsimd SWDGE queue so the
            # HWDGE FIFOs never make V-proj operands late.  Contiguous
            # whole-tile weight DMAs (2KB rows) -- a pair-split strided DMA
            # (256B descriptors) transfers 4x slower and was on the
            # critical path.
            nc.sync.dma_start(wq_sb[:], WQ[:])
            nc.scalar.dma_start(wk_sb[:], WK[:])
            for c in range(CT):
                q = nc.sync if c % 2 == 0 else nc.scalar
                q.dma_start(xt_t[c][:, 0:1024], XT[c * 128 : (c + 1) * 128, 0:1024])
            for c in range(CT):
                nc.gpsimd.dma_start(wv_t[c][:], WV[c * 128 : (c + 1) * 128, :])
            for c in range(CT):
                q = nc.sync if c % 2 == 0 else nc.scalar
                q.dma_start(xt_t[c][:, 1024:2048], XT[c * 128 : (c + 1) * 128, 1024:2048])
            mv_t = p_c.tile([128, KT + 8], F32, tag="mv", name="mv_t")
            nc.gpsimd.dma_start(mv_t[:, 0:KT], MV[:])
            # ones scratch for the V ones-column (written once)
            nc.vector.memset(mv_t[:, KT : KT + 4], 1.0)
            ones64 = p_c.tile([1, 64], F32, tag="ones64", name="ones64")
            nc.vector.memset(ones64[0:1, 0:64], 1.0)
            # dummy exp to pull the ScalarE ACT-table load (~2.7us) into the
            # DMA lead-in instead of delaying the first real exp
            nc.scalar.activation(
                mv_t[0:1, KT + 5 : KT + 6], mv_t[0:1, KT : KT + 1], EXP
            )
            wo_t = []
            for p in range(2):
                t = p_wo.tile([128, C], BF16, tag="wo", name="wo_t")
                nc.gpsimd.dma_start(t[:], WO[p * 128 : (p + 1) * 128, :])
                wo_t.append(t)

            # ---------------- QKV projection ----------------
            qk_tiles = {}

            def proj_qk_half(nm, wsb, pair, half, warmup=False):
                key = (nm, pair)
                if key not in qk_tiles:
                    qk_tiles[key] = p_qk.tile([128, S], BF16, tag="qk", name="qk_t")
                dst = qk_tiles[key]
                off = half * 1024
                pst = ps.tile([128, 1024], F32, tag="st", name="pp")
                for c in range(CT):
                    wt = wsb[:, c * 256 + pair * 128 : c * 256 + (pair + 1) * 128]
                    for n in range(2):
                        nc.tensor.matmul(
                            pst[:, n * 512 : (n + 1) * 512],
                            lhsT=wt,
                            rhs=xt_t[c][:, off + n * 512 : off + (n + 1) * 512],
                            start=(c == 0),
                            stop=(c == CT - 1),
                        )
                    if warmup and 1 <= c <= 6:
                        # discarded matmuls inside the DMA-wait gaps of the
                        # lead-in: accumulate HAM busy-time so the PE clock
                        # un-throttles before the attention stream starts
                        if not wup_holder:
                            wup_holder.append(
                                ps.tile([128, 512], F32, tag="inj", name="wup")
                            )
                        nc.tensor.matmul(
                            wup_holder[0][:, 0:512],
                            lhsT=wsb[:, 0:128],
                            rhs=xt_t[0][:, 0:512],
                            start=True,
                            stop=True,
                        )
                nc.vector.tensor_copy(dst[:, off : off + 1024], pst[:, 0:1024])

            def proj_qk_quarter(nm, pair, quarter, tag="inj", evict_scalar=False):
                wsb = wq_sb if nm == "q" else wk_sb
                key = (nm, pair)
                if key not in qk_tiles:
                    qk_tiles[key] = p_qk.tile([128, S], BF16, tag="qk", name="qk_t")
                dst = qk_tiles[key]
                off = quarter * 512
                pst = ps.tile([128, 512], F32, tag=tag, name="pq")
                for c in range(CT):
                    wt = wsb[:, c * 256 + pair * 128 : c * 256 + (pair + 1) * 128]
                    nc.tensor.matmul(
                        pst[:, 0:512],
                        lhsT=wt,
                        rhs=xt_t[c][:, off : off + 512],
                        start=(c == 0),
                        stop=(c == CT - 1),
                    )
                if evict_scalar:
                    # loops 1/2 inject quarters at k=9,11 where the DVE is
                    # congested (co-exps + normalize); ScalarE has measured
                    # gap slack exactly there
                    nc.scalar.copy(dst[:, off : off + 512], pst[:, 0:512])
                else:
                    nc.vector.tensor_copy(dst[:, off : off + 512], pst[:, 0:512])

            v_t = []

            def proj_v_tile(st):
                psv = ps.tile([128, 512], F32, tag="inj", name="psv")
                for c in range(CT):
                    nc.tensor.matmul(
                        psv[:, 0 : HL * DQ],
                        lhsT=xt_t[c][:, st * 128 : (st + 1) * 128],
                        rhs=wv_t[c][:],
                        start=(c == 0),
                        stop=(c == CT - 1),
                    )
                vt = p_v.tile([128, HL * 65], BF16, tag="v", name="v_t")
                v3 = vt[:, 0 : HL * 65].rearrange("p (h c) -> p h c", c=65)
                s3 = psv[:, 0 : HL * DQ].rearrange("p (h c) -> p h c", c=DQ)
                o3 = mv_t[:, KT : KT + 4].rearrange("p (h c) -> p h c", c=1)
                # fused eviction+mask: V cols straight from PSUM * mask, and
                # the ones-column (rowsum trick) = 1.0 * mask
                nc.vector.tensor_scalar_mul(
                    v3[:, :, 0:DQ], s3[:, :, :], mv_t[:, st : st + 1]
                )
                nc.vector.tensor_scalar_mul(
                    v3[:, :, DQ : DQ + 1], o3[:, :, :], mv_t[:, st : st + 1]
                )
                v_t.append(vt)

            wup_holder = []
            # Lead-in: ONLY the half-0 projections (pair-0 wq/wk + xt-half0,
            # ~2.5MB of DMA), with Q and K INTERLEAVED per c-chunk so each
            # chunk's 4 matmuls run as soon as that chunk's DMA lands (the
            # in-order tensor queue would otherwise hold all K matmuls
            # behind Q's last chunk).  Discarded warmup matmuls fill the
            # DMA-wait gaps to accumulate HAM busy-time (PE ramps to full
            # clock before the attention stream starts).  xt-half1 lands at
            # ~25us so the half-1 and pair-1 projections are injected inside
            # the attention loops instead.
            def leadin_qk_h0():
                for nm in ("q", "k"):
                    qk_tiles[(nm, 0)] = p_qk.tile([128, S], BF16, tag="qk", name="qk_t")
                pq = ps.tile([128, 1024], F32, tag="st", name="pq0")
                pk = ps.tile([128, 1024], F32, tag="st", name="pk0")
                for c in range(CT):
                    for pst, wsb in ((pq, wq_sb), (pk, wk_sb)):
                        wt = wsb[:, c * 256 : c * 256 + 128]
                        for n in range(2):
                            nc.tensor.matmul(
                                pst[:, n * 512 : (n + 1) * 512],
                                lhsT=wt,
                                rhs=xt_t[c][:, n * 512 : (n + 1) * 512],
                                start=(c == 0),
                                stop=(c == CT - 1),
                            )
                # parallel evictions: q on DVE, k on ScalarE (its DMA
                # issues are done by now; the first exp isn't ready yet)
                nc.vector.tensor_copy(qk_tiles[("q", 0)][:, 0:1024], pq[:, 0:1024])
                nc.scalar.copy(qk_tiles[("k", 0)][:, 0:1024], pk[:, 0:1024])

            leadin_qk_h0()

            # -------- attention: flattened 4-loop software pipeline --------
            # Loop li = pair*2 + j.  Per loop the St/exp stream runs k=0..15
            # on the st tag (rotation paced ONLY by the exp drain).  Head-A
            # PV trails St by PIPE_A on the accA tag; head-B PV starts at
            # SB[li] (its accumulator halves are the LAST inj-tag allocs of
            # the loop, reading pt tiles buffered in SBUF) and its remainder
            # drains 3-per-iteration into the next loop, followed by the
            # accumulator evictions and the normalize chain -- the exp
            # stream never pauses at a j-boundary.  Injected work (V-proj,
            # quarter-projs, out-proj halves, bcast tiles) rides the inj tag
            # between the head-B claims and so never displaces a score tile.
            PIPE_A = 2
            SB = {0: 13, 1: 12, 2: 12, 3: 13}
            # DVE exps sit on head-B tiles (their consumer, head-B PV, runs
            # far later so the 2.44us latency never blocks anything) and are
            # kept AWAY from the k-window where the previous loop's normalize
            # chain occupies the in-order DVE queue (k<=8) -- an exp queued
            # behind 3us of normalize work delays the st-slot rotation.
            # In the PE-thin k=12..15 stretch the loops are ScalarE-paced at
            # ~77%% PE duty, which trips the HAM half-clock gate.  A DVE
            # co-exp every other k there raises the exp drain rate until the
            # PE is the pacer at full duty -- no idle window, no gate.
            DVE_EXP_LI = {
                0: {12, 14},
                1: {9, 11, 12, 14},
                2: {9, 11, 12, 14},
                3: {9, 12, 14},
            }
            # discarded extra score-matmul passes (same psum tile, start=True
            # resets) in the PE-thin k-slots: keeps PE duty >=80% so the HAM
            # clock-gate never drops the PE to half rate mid-pipeline.
            # value = how many heads get a discarded extra pass at that k.
            # NOTE: discarded-matmul filler (doubled-St style) at scale trips
            # a DEVICE power throttle to 2.0GHz (two 301us runs, every op
            # stretched exactly 1.2x) -- real work only.
            # surgical: loop-3 k=5..7 is the one PE-thin stretch (St+pvA
            # only, ~39% duty) that the DVE co-exp trick cannot densify --
            # it stochastically trips a 34us HAM half-clock window over
            # loop 3 + tail (measured 233us draws vs the 217us cluster).
            # 12 discarded matmuls (~1.5% extra PE power) pin it warm; the
            # device power throttle needed ~15%-scale filler to trip.
            ST_REPS = {3: {5: 2, 6: 2, 7: 2}}
            pts = {}
            accA = {}
            accB = {}
            ot_tiles = {}
            evs_store = {}
            rs_tiles = {}
            rc_tiles = {}
            bct_tiles = {}

            def emit_exp(pt, stp, on_dve):
                if on_dve:
                    h = p_h.tile([128, 1024], F32, tag="h", name="h_t")
                    nc.vector._custom_dve(
                        EXP_CUBIC, out=h[:], in0=stp[:],
                        s0=EXPC_S0, s1=EXPC_S1, imm2=EXPC_IMM2,
                    )
                    nc.vector._custom_dve(POW64, out=pt[:], in0=h[:])
                else:
                    nc.scalar.activation(pt[:], stp[:], EXP, scale=1.0 / SCALE)

            def st_exp(li, k):
                pair, j = divmod(li, 2)
                qt = qk_tiles[("q", pair)]
                kt = qk_tiles[("k", pair)]
                jo = j * 1024
                dve_ks = DVE_EXP_LI.get(li, set())
                reps = ST_REPS.get(li, {}).get(k, 0)
                for i, base in enumerate((0, 64)):
                    stp = ps.tile([128, 1024], F32, tag="st", name="stp")
                    for rep in range(1 + (1 if i < reps else 0)):
                        for n in range(2):
                            nc.tensor.matmul(
                                stp[:, n * 512 : (n + 1) * 512],
                                lhsT=kt[base : base + DQ, k * 128 : (k + 1) * 128],
                                rhs=qt[base : base + DQ, jo + n * 512 : jo + (n + 1) * 512],
                                start=True,
                                stop=True,
                            )
                    pt = p_pt.tile(
                        [128, 1024], BF16,
                        tag=("ptA" if i == 0 else "ptB"),
                        bufs=(6 if i == 0 else 16),
                        name="pt_t",
                    )
                    emit_exp(pt, stp, on_dve=(i == 1 and k in dve_ks))
                    pts[(li, k, i)] = pt

            def pv(li, k, i):
                pair, j = divmod(li, 2)
                h = 2 * pair + i
                accs = accA if i == 0 else accB
                if k == 0:
                    accs[li] = [
                        ps.tile([65, 512], F32,
                                tag=("accA" if i == 0 else "inj"), name="acc")
                        for _ in range(2)
                    ]
                pt = pts.pop((li, k, i))
                for n in range(2):
                    nc.tensor.matmul(
                        accs[li][n][0:65, 0:512],
                        lhsT=v_t[k][:, h * 65 : h * 65 + 65],
                        rhs=pt[:, n * 512 : (n + 1) * 512],
                        start=(k == 0),
                        stop=(k == KT - 1),
                    )

            def evict_acc(li, i, hopq=None):
                # acc halves -> ev [65,1024] f32; rowsum row 64 DMA-hops to
                # partition 0 of the loop's [1,2048] rs tile.  At the tail
                # the hops ride the idle gpsimd queue -- the sync queue is
                # backlogged with output DMAs and would delay the recip.
                accs = (accA if i == 0 else accB).pop(li)
                ev = p_ev.tile([65, 1024], F32, tag="ev", name="ev_t")
                if li not in rs_tiles:
                    rs_tiles[li] = p_rs.tile([1, 2048], F32, tag="rs", name="rs_t")
                rs = rs_tiles[li]
                hopq = hopq or nc.sync
                for n in range(2):
                    nc.vector.tensor_copy(
                        ev[0:65, n * 512 : (n + 1) * 512], accs[n][0:65, 0:512]
                    )
                    hopq.dma_start(
                        rs[0:1, i * 1024 + n * 512 : i * 1024 + (n + 1) * 512],
                        ev[64:65, n * 512 : (n + 1) * 512],
                    )
                evs_store[(li, i)] = ev

            def norm_recip(li, i):
                # per-head [1,1024] reciprocal, spread across two k-slots so
                # the in-order DVE queue never holds >1.2us of normalize work
                if li not in rc_tiles:
                    rc_tiles[li] = p_rc.tile([1, 2048], F32, tag="rc", name="rc_t")
                rc = rc_tiles[li]
                nc.vector.reciprocal_approx_fast(
                    rc[0:1, i * 1024 : (i + 1) * 1024],
                    rs_tiles[li][0:1, i * 1024 : (i + 1) * 1024],
                )
                if i == 1:
                    rs_tiles.pop(li)

            def norm_bcast(li):
                # mid-loop broadcast of 1/rowsum on the otherwise-idle
                # gpsimd: the PE-matmul broadcast would sit behind ~12
                # k-iterations of queued matmuls, and the DVE multiplies
                # waiting on it fill the DVE 4-deep wait queue and block the
                # k=12/14 co-exps (measured 8.5us exp gap).
                bct = p_bc.tile([64, 2048], F32, tag="bc", name="bc_t")
                nc.gpsimd.partition_broadcast(bct[0:64, 0:2048], rc_tiles[li][0:1, 0:2048])
                bct_tiles[li] = bct

            def norm_apply_bct(li, i):
                if li not in ot_tiles:
                    ot_tiles[li] = p_ot.tile([128, 1024], BF16, tag="ot", name="ot_t")
                ot = ot_tiles[li]
                bct = bct_tiles[li]
                ev = evs_store.pop((li, i))
                nc.vector.tensor_mul(
                    ot[i * 64 : (i + 1) * 64, 0:1024],
                    ev[0:64, 0:1024],
                    bct[0:64, i * 1024 : (i + 1) * 1024],
                )
                if i == 1:
                    rc_tiles.pop(li)
                    bct_tiles.pop(li)

            def norm_apply_head(li, i):
                # O = PV/rowsum via PE partition-broadcast of 1/rowsum (tiny
                # ones[1,64]^T @ rc matmuls into inj-tag psum) + DVE multiply;
                # head A -> ot rows 0-63, head B -> rows 64-127.  (gpsimd
                # cannot read PSUM, so the multiply must stay on DVE.)
                if li not in ot_tiles:
                    ot_tiles[li] = p_ot.tile([128, 1024], BF16, tag="ot", name="ot_t")
                ot = ot_tiles[li]
                rc = rc_tiles[li]
                ev = evs_store.pop((li, i))
                for n in range(2):
                    bcp = ps.tile([64, 512], F32, tag="inj", name="bcp")
                    nc.tensor.matmul(
                        bcp[0:64, 0:512],
                        lhsT=ones64[0:1, 0:64],
                        rhs=rc[0:1, i * 1024 + n * 512 : i * 1024 + (n + 1) * 512],
                        start=True,
                        stop=True,
                    )
                    nc.vector.tensor_mul(
                        ot[i * 64 : (i + 1) * 64, n * 512 : (n + 1) * 512],
                        ev[0:64, n * 512 : (n + 1) * 512],
                        bcp[0:64, 0:512],
                    )
                if i == 1:
                    rc_tiles.pop(li)

            def norm_apply(li, tail=False):
                norm_apply_head(li, 0)
                norm_apply_head(li, 1)

            def outproj_half(st, n, tail=False):
                po = ps.tile([128, 512], F32, tag="inj", name="po")
                sj, so = st // 8, (st % 8) * 128
                for p in range(2):
                    nc.tensor.matmul(
                        po[:, 0:512],
                        lhsT=ot_tiles[p * 2 + sj][:, so : so + 128],
                        rhs=wo_t[p][:, n * 512 : (n + 1) * 512],
                        start=(p == 0),
                        stop=(p == 1),
                    )
                os_t = p_os.tile([128, 512], BF16, tag="os", name="os_t")
                if tail and n == 1:
                    nc.scalar.copy(os_t[:, 0:512], po[:, 0:512])
                else:
                    nc.vector.tensor_copy(os_t[:, 0:512], po[:, 0:512])
                # mid-loop output DMAs stay off the scalar queue (a
                # scalar.dma_start would block the exp stream); the tail
                # stripes across both HWDGE paths
                qd = nc.scalar if (tail and (st + n) % 2 == 1) else nc.sync
                qd.dma_start(
                    OUT[st * 128 : (st + 1) * 128, n * 512 : (n + 1) * 512],
                    os_t[:, 0:512],
                )

            # per-loop event schedule: events[li][k] = thunks run AFTER
            # st_exp(li, k).  Build order matters within a k-slot: PV trails
            # and drains first, then acc evictions, then injections (an inj
            # alloc emitted before the eviction that frees its slot would
            # head-of-line-block the tensor queue).
            events = [defaultdict(list) for _ in range(4)]

            def at(li, k, th):
                events[li][k].append(th)

            mk = lambda f, *a, **kw: (lambda: f(*a, **kw))

            for li in range(4):
                for k in range(KT):
                    if k - PIPE_A >= 0:
                        at(li, k, mk(pv, li, k - PIPE_A, 0))
                    if k - SB[li] >= 0:
                        at(li, k, mk(pv, li, k - SB[li], 1))
                if li < 3:
                    at(li + 1, 0, mk(pv, li, 14, 0))
                    at(li + 1, 1, mk(pv, li, 15, 0))
                    rem = list(range(KT - SB[li], KT))
                    dk = 0
                    while rem:
                        for b in rem[:3]:
                            at(li + 1, dk, mk(pv, li, b, 1))
                        rem = rem[3:]
                        dk += 1
                    at(li + 1, 2, mk(evict_acc, li, 0))
                    at(li + 1, 3, mk(norm_recip, li, 0))
                    at(li + 1, dk, mk(evict_acc, li, 1))
                    at(li + 1, dk + 1, mk(norm_recip, li, 1))
                    at(li + 1, dk + 2, mk(norm_bcast, li))
                    at(li + 1, dk + 3, mk(norm_apply_bct, li, 0))
                    at(li + 1, dk + 4, mk(norm_apply_bct, li, 1))

            # loop-0 injections: V-proj (16 tiles; wv lands ~20us so st0
            # waits for k=1; st8+ read xt-half1, at k>=7), K0-half1 quarters
            # (deadlines k=8/k=12), Q0-half1 via st-tag displacement late in
            # the loop (it is PE-bound there; the exp stream has slack)
            for kk, sts in {0: (0, 2), 1: (2, 4), 2: (4, 6), 3: (6, 8),
                            4: (8, 10), 7: (10, 12), 8: (12, 14),
                            9: (14, 16)}.items():
                for st in range(*sts):
                    at(0, kk, mk(proj_v_tile, st))
            at(0, 5, mk(proj_qk_quarter, "k", 0, 2))
            at(0, 6, mk(proj_qk_quarter, "k", 0, 3))
            at(0, 10, mk(proj_qk_quarter, "q", 0, 2))
            at(0, 11, mk(proj_qk_quarter, "q", 0, 3))
            # loop-1 injections: pair-1 Q/K half-0 quarters (needed by
            # loop 2), spread k=5..11 to fill the PE-thin post-drain region
            at(1, 5, mk(proj_qk_quarter, "q", 1, 0))
            at(1, 7, mk(proj_qk_quarter, "q", 1, 1))
            # k=8/10, NOT 9/11: a quarter on the same k as a DVE co-exp
            # stacks 3us on the DVE (exp pair + evict) while the quarter's
            # matmuls overload the PE -- measured 1.9+1.7us exp gaps per loop
            at(1, 8, mk(proj_qk_quarter, "k", 1, 0))
            at(1, 10, mk(proj_qk_quarter, "k", 1, 1))
            # loop-2 injections: pair-1 half-1 quarters (K needed by its own
            # k=8/k=12, Q by loop 3)
            at(2, 5, mk(proj_qk_quarter, "k", 1, 2))
            at(2, 7, mk(proj_qk_quarter, "k", 1, 3))
            at(2, 8, mk(proj_qk_quarter, "q", 1, 2))
            at(2, 10, mk(proj_qk_quarter, "q", 1, 3))
            # loop-3 injections: out-proj s-slices 0-4 (need ot10, ready ~k8)
            for idx, st in enumerate(range(0, 5)):
                at(3, 8 + idx, mk(outproj_half, st, 0))
                at(3, 8 + idx, mk(outproj_half, st, 1))

            for li in range(4):
                for k in range(KT):
                    st_exp(li, k)
                    for th in events[li][k]:
                        th()

            # ---------------- tail: loop-3 drain + out-proj 5-15 ------------
            # Scores are done, so the 4-bank st tag is repurposed for FULL
            # [128,1024] out-proj slices (2-slot ping-pong, evictions split
            # DVE/ScalarE in parallel) -- the 1-bank inj rotation would pace
            # at ~1.2us/half-slice.  Slices 5-7 depend only on ot00/ot10, so
            # their matmuls interleave with the head-B drain and keep the PE
            # warm through the normalize chain.
            def outproj_slice_tail(st):
                # even slices: one [128,1024] st-tag psum; odd slices: two
                # [128,512] accA-tag halves -> a 4-deep slot pipeline
                sj, so = st // 8, (st % 8) * 128
                os_t = p_os.tile([128, 1024], BF16, tag="osf", bufs=4, name="os_t")
                if st % 2 == 0:
                    full = ps.tile([128, 1024], F32, tag="st", name="po")
                    dsts = [full[:, 0:512], full[:, 512:1024]]
                else:
                    dsts = [
                        ps.tile([128, 512], F32, tag="accA", name="poh")[:, 0:512]
                        for _ in range(2)
                    ]
                for n in range(2):
                    for p in range(2):
                        nc.tensor.matmul(
                            dsts[n],
                            lhsT=ot_tiles[p * 2 + sj][:, so : so + 128],
                            rhs=wo_t[p][:, n * 512 : (n + 1) * 512],
                            start=(p == 0),
                            stop=(p == 1),
                        )
                    if n == 0:
                        nc.vector.tensor_copy(os_t[:, 0:512], dsts[0])
                    else:
                        nc.scalar.copy(os_t[:, 512:1024], dsts[1])
                    qd = nc.sync if n == 0 else nc.scalar
                    qd.dma_start(
                        OUT[st * 128 : (st + 1) * 128, n * 512 : (n + 1) * 512],
                        os_t[:, n * 512 : (n + 1) * 512],
                    )

            pv(3, 14, 0)
            pv(3, 15, 0)
            evict_acc(3, 0)
            norm_recip(3, 0)
            for b in range(KT - SB[3], KT):
                pv(3, b, 1)
            evict_acc(3, 1)
            norm_recip(3, 1)
            norm_apply_head(3, 0)
            outproj_slice_tail(5)
            outproj_slice_tail(6)
            # 3 discarded matmuls bridge the recipB->bcpB latency: the ~2us
            # PE hole here is what trips the tail half-clock window
            brid = ps.tile([128, 512], F32, tag="st", name="brid")
            for _ in range(3):
                nc.tensor.matmul(
                    brid[:, 0:512], lhsT=wq_sb[:, 0:128], rhs=wq_sb[:, 0:512],
                    start=True, stop=True,
                )
            norm_apply_head(3, 1)
            outproj_slice_tail(7)
            for st in range(8, KT):
                outproj_slice_tail(st)


# revision 39
# speedup vs baseline: 1.0041x; 1.0041x over previous
"""Multi-head attention kernel for Trainium2, SPMD across 8 NeuronCores.

Problem: b=2, s=2048, d_model=1024, 16 heads x 64 dims, packed QKV proj,
softmax over keys (boolean key mask), out-projection.

Sharding: core c in 0..7 handles batch b = c//4 and a group of 4 heads
g = c%4 (data parallel over batch x head/tensor parallel).  Each core
computes its head-group's out-projection partial [2048, 1024]; the host
sums the 4 partials per batch (the row-parallel reduction) and upcasts
from bf16.

Architecture: a FLATTENED 4-loop software pipeline (loop li = pair*2 +
sq-half), one continuous St/exp stream across all loop boundaries.

PSUM (8 banks, one pool, three independently-rotating tags):
  st   2x [128,1024] = 4 banks: score tiles ONLY.  The rotation is paced
       purely by the exp drain, so injected work can never displace a
       score tile (the baseline's single shared rotation cost ~50us of
       exp-stream stalls).
  accA 2x [65,512] halves = 2 banks: head-A PV accumulator, resident a
       whole loop.
  inj  2x [128,512] = 2 banks: serial injection stream (V-proj, Q/K
       quarter-projections, out-proj halves, PE-broadcast tiles) and the
       head-B PV accumulator halves, claimed at k=SB[li] as the LAST inj
       allocs of the loop.

Split-head PV: head-A PV trails St by 2; head-B PV starts at SB[li]
(~12) reading pt tiles buffered in SBUF (ptA tag 6 bufs, ptB 16), and
its remainder drains 3-per-iteration into the NEXT loop's first k's,
followed by the accumulator evictions and the normalize chain -- the
exp stream never pauses at a j-boundary.

exp: ScalarE activation Exp (scale=1/8, ~1.11us per [128,1024] tile) +
DVE co-exps (custom 2-op cubic^64, 2.44us/pair) on head-B tiles per
DVE_EXP_LI.  Placement is the key lever: in the PE-thin k=12..15 tail
of each loop the pipeline is ScalarE-paced at ~77% PE duty, which trips
the HAM half-clock gate; a DVE co-exp every other k there makes the
stretch PE-paced at full duty (no idle window, no gate), and co-exps on
the quarter-projection k's absorb their PE load.  Co-exps must stay
clear of k<=8 where the previous loop's normalize chain occupies the
in-order DVE queue.

normalize (mid-loop): acc halves evict to SBUF (DVE), rowsum rows
DMA-hop to one [1,2048] tile, per-head reciprocals, then gpsimd
partition_broadcast + DVE multiplies.  The broadcast MUST be gpsimd
here: a PE-matmul broadcast sits behind ~12 k-iterations of queued
matmuls and the DVE multiplies waiting on it fill the DVE's 4-deep
wait queue, blocking the co-exps (measured 8.5us/boundary).  gpsimd
runs ONLY partition_broadcast + input SWDGE DMAs (no ucode thrash).
At the TAIL the broadcast is tiny PE matmuls instead (PSUM free, PE
idle); the rowsum hops stay on sync HWDGE (gpsimd SWDGE descriptor
generation costs ~1.3us/DMA and delayed the recip by ~3.5us), and a
3-matmul discarded bridge after slice 6 covers the recipB latency so
the PE never idles into the half-clock gate.

out-proj: slices 0-4 injected into loop 3 (k=8..12, inj tag); slices
5-15 in the tail on the freed st+accA tags (4-deep slot pipeline;
evictions split DVE/ScalarE, output DMAs striped sync/scalar).  Slices
5-7 need only the j0 ot tiles so their matmuls bridge the head-B
normalize chain.

Input DMA: issues striped sync/scalar in exact consumption order
(pair-0 wq | wk halves, xt-half0 alternating, xt-half1, pair-1 weight
halves), non-critical wv/mask/wo on
the gpsimd SWDGE queue.  Lead-in projects ONLY pair-0 Q/K half-0,
Q and K interleaved per c-chunk so each chunk's matmuls fire as its
DMA lands; half-1 and pair-1 projections are injected into the loops
(quarters, deadlines per consumption).  A dummy 1-element exp pulls
the ACT-table load into the DMA lead-in.

Measured: 217.6us at full clock, HAM warm end-to-end (baseline 253.5us; throttled
runs read ~258us).  Without the loop-3 k=5..7 ST_REPS filler the HAM
gate stochastically costs a 34us half-clock window there (233us draws);
the 12-matmul filler pins it warm at ~1.5% extra PE power.  Known HW traps:
discarded-matmul filler at scale trips a DEVICE power throttle to
2.0GHz (everything stretches exactly 1.2x -- that throttle also
appears stochastically on some runs regardless); gpsimd cannot read
PSUM; scalar.dma_start blocks the exp stream; 1-lane DVE ops based at
partition 64 return garbage; fp32/bf16 matmul operand mixing is
rejected; an inj-tag alloc emitted after the accB claim of the same
loop head-of-line-blocks the tensor queue until the next loop's drain.
"""

import numpy as np
import ml_dtypes

BF = ml_dtypes.bfloat16
S = 2048
C = 1024
DQ = 64
HL = 4  # local heads per core
KT = S // 128  # 16 key tiles
CT = C // 128  # 8 contraction tiles
SCALE = 8.0  # sqrt(DQ)

# exp(s/8) = p3(s/512)^64, p3 = cubic Taylor of e^v at 0
_R = 1.0 / 512.0
EXPC_S0 = _R * _R * _R / 6.0  # v^3 coeff (on raw scores)
EXPC_S1 = _R * _R / 2.0  # v^2 coeff
EXPC_IMM2 = _R  # v^1 coeff
# (k, head) pairs whose exp runs on DVE, per (pair, j).  ONE head per k
# (the other head's exp stays on ScalarE in parallel), kept away from the
# first/last few k so the j-boundary normalize chain and the psA ping-pong
# restart never queue behind slow DVE exps (2 x 1.22us each).  Only the
# loops without injected PE filler get DVE exps -- the padded loops are
# PE-bound and ScalarE alone keeps up.
# pair-1 j0 also carries the double-St density guard: 8 DVE exps there
# compound with the doubled score matmuls into a 2.9us/k crawl -- 4 is the
# sweet spot (measured).
DVE_EXP_KI = {
    (1, 0): {(5, 1), (7, 0), (9, 1), (11, 0)},
    (1, 1): {(5, 1), (7, 0), (9, 1), (11, 0)},
}

_CACHED = None
_DVE_OPS = None


def _register_dve_ops():
    """Register the two custom DVE exp uOps into concourse's per-process op
    table (the repo is read-only, so dve_ops.py can't be edited; appending
    to the module-level registry at runtime is equivalent -- the per-NEFF
    DVE table generator and the ISA row lookup both read these dicts)."""
    global _DVE_OPS
    if _DVE_OPS is not None:
        return _DVE_OPS
    import concourse.dve_ops as dve_ops
    from concourse.dve_ops import DveOp
    from concourse.dve_spec import Spec, Src0, C0, C1, C2, One, sq, lower
    from concourse.dve_uop import DveOpSpec

    def ref_cubic(in0, in1, s0, s1, imm2):
        x = in0.astype(np.float32)
        return (
            (np.float32(s0) * x + np.float32(s1)) * x + np.float32(imm2)
        ) * x + np.float32(1.0)

    def ref_pow64(in0, in1, s0, s1, imm2):
        x = in0.astype(np.float32)
        for _ in range(6):
            x = (x * x).astype(np.float32)
        return x

    body1 = ((C0 * Src0 + C1) * Src0 + C2) * Src0 + One
    b = Src0
    for _ in range(6):
        b = sq(b)
    specs = [
        ("EXP_CUBIC_ANTK", Spec(body=body1, reference=ref_cubic)),
        ("POW64_ANTK", Spec(body=b, reference=ref_pow64)),
    ]
    ops = []
    for name, spec in specs:
        if name in dve_ops._SUB_OPCODE_FOR_NAME:
            ops.append(next(o for o in dve_ops.OPS if o.name == name))
            continue
        row = dve_ops._CUSTOM_DVE_ROW_BASE + len(dve_ops.OPS)
        assert row < 0x20
        shas = {}
        for ver in ("v3", "v4"):
            s = DveOpSpec(
                name=name, opcode=row, uops=lower(spec, ver=ver), rd1_en=False
            )
            shas[ver] = s.sha(ver)
        op = DveOp(name, spec, subdim=False, uops_sha=shas)
        dve_ops.OPS.append(op)
        dve_ops._SUB_OPCODE_FOR_NAME[name] = row
        dve_ops.CUSTOM_DVE_SPECS[name] = spec
        ops.append(op)
    _DVE_OPS = tuple(ops)
    return _DVE_OPS


def _build():
    import concourse.bacc as bacc
    import concourse.mybir as mybir
    import concourse.tile as tile

    EXP_CUBIC, POW64 = _register_dve_ops()

    F32 = mybir.dt.float32
    BF16 = mybir.dt.bfloat16
    EXP = mybir.ActivationFunctionType.Exp

    nc = bacc.Bacc(
        "TRN2",
        target_bir_lowering=False,
        debug=False,
        enable_asserts=False,
        num_devices=8,
    )

    XT = nc.dram_tensor("xt", [C, S], BF16, kind="ExternalInput").ap()
    WQ = nc.dram_tensor("wq", [128, CT * 256], BF16, kind="ExternalInput").ap()
    WK = nc.dram_tensor("wk", [128, CT * 256], BF16, kind="ExternalInput").ap()
    WV = nc.dram_tensor("wv", [C, 2 * 128], BF16, kind="ExternalInput").ap()
    WO = nc.dram_tensor("wo", [HL * DQ, C], BF16, kind="ExternalInput").ap()
    MV = nc.dram_tensor("maskv", [128, KT], F32, kind="ExternalInput").ap()
    OUT = nc.dram_tensor("out", [S, C], BF16, kind="ExternalOutput").ap()

    from contextlib import ExitStack
    from collections import defaultdict

    with tile.TileContext(nc) as tc:
        with ExitStack() as stack:
            pool = lambda *a, **k: stack.enter_context(tc.tile_pool(*a, **k))
            p_xt = pool(name="xt", bufs=CT)
            p_w = pool(name="wqk", bufs=2)
            p_wv = pool(name="wv", bufs=CT)
            p_wo = pool(name="wo", bufs=2)
            p_c = pool(name="cst", bufs=1)
            p_qk = pool(name="qk", bufs=4)
            p_v = pool(name="v", bufs=KT)
            p_pt = pool(name="pt", bufs=2)
            p_h = pool(name="h", bufs=2)
            p_ev = pool(name="ev", bufs=2)
            p_rc = pool(name="rc", bufs=2)
            p_rs = pool(name="rs", bufs=2)
            p_ot = pool(name="ot", bufs=4)
            p_bc = pool(name="bc", bufs=1)
            p_os = pool(name="os", bufs=8)
            # ONE psum pool, tag-partitioned (tags rotate independently):
            #   st   2x [128,1024] f32 = 4 banks -- score tiles ONLY; the
            #        rotation is paced purely by the exp drain, so the exp
            #        stream can never be displaced by injected work
            #   accA 2x [65,512]   f32 = 2 banks -- head-A PV accumulator
            #        (n-halves), resident for a whole j-loop
            #   inj  2x [128,512]  f32 = 2 banks -- serial injection stream:
            #        V-proj / Q,K quarter-projs / out-proj halves / bcast
            #        tiles, and the head-B PV accumulator halves at loop end
            ps = pool(name="ps", bufs=2, space="PSUM")
            # ---------------- input DMA ----------------
            # The DMA ISSUE cost is ~600ns per instruction on a queue
            # regardless of size (cost ~ partition-row count), and there are
            # two HWDGE queues (sync + scalar).  The baseline put 43 issues
            # on sync alone = ~26us of serialized issue -- the first exp
            # waited 29us on it.  Stripe the issues across BOTH queues in
            # exact consumption order (Q-half0 needs wq + xt-half0, K-half0
            # adds wk, V needs wv + xt-half0 cols, half1 traffic last), and
            # merge each weight into ONE whole-tile DMA.
            wq_sb = p_w.tile([128, CT * 256], BF16, tag="wq", name="wq_sb")
            wk_sb = p_w.tile([128, CT * 256], BF16, tag="wk", name="wk_sb")
            xt_t = [p_xt.tile([128, S], BF16, tag="xt", name="xt_t") for _ in range(CT)]
            wv_t = []
            for c in range(CT):
                wv_t.append(p_wv.tile([128, HL * DQ], BF16, tag="wv", name="wv_t"))
            # critical stream (wq + wk + xt-half0 + xt-half1) striped across
            # the two HWDGE queues; ALL non-critical input (wv, mask, wo)
            # goes through the otherwise-idle gpsimd SWDGE queue so the
            # HWDGE FIFOs never make V-proj operands late.  Contiguous
            # whole-tile weight DMAs (2KB rows) -- a pair-split strided DMA
            # (256B descriptors) transfers 4x slower and was on the
            # critical path.
            nc.sync.dma_start(wq_sb[:], WQ[:])
            nc.scalar.dma_start(wk_sb[:], WK[:])
            for c in range(CT):
                q = nc.sync if c % 2 == 0 else nc.scalar
                q.dma_start(xt_t[c][:, 0:1024], XT[c * 128 : (c + 1) * 128, 0:1024])
            for c in range(CT):
                nc.gpsimd.dma_start(wv_t[c][:], WV[c * 128 : (c + 1) * 128, :])
            for c in range(CT):
                q = nc.sync if c % 2 == 0 else nc.scalar
                q.dma_start(xt_t[c][:, 1024:2048], XT[c * 128 : (c + 1) * 128, 1024:2048])
            mv_t = p_c.tile([128, KT + 8], F32, tag="mv", name="mv_t")
            nc.gpsimd.dma_start(mv_t[:, 0:KT], MV[:])
            # ones scratch for the V ones-column (written once)
            nc.vector.memset(mv_t[:, KT : KT + 4], 1.0)
            ones64 = p_c.tile([1, 64], F32, tag="ones64", name="ones64")
            nc.vector.memset(ones64[0:1, 0:64], 1.0)
            # dummy exp to pull the ScalarE ACT-table load (~2.7us) into the
            # DMA lead-in instead of delaying the first real exp
            nc.scalar.activation(
                mv_t[0:1, KT + 5 : KT + 6], mv_t[0:1, KT : KT + 1], EXP
            )
            wo_t = []
            for p in range(2):
                t = p_wo.tile([128, C], BF16, tag="wo", name="wo_t")
                nc.gpsimd.dma_start(t[:], WO[p * 128 : (p + 1) * 128, :])
                wo_t.append(t)

            # ---------------- QKV projection ----------------
            qk_tiles = {}

            def proj_qk_half(nm, wsb, pair, half, warmup=False):
                key = (nm, pair)
                if key not in qk_tiles:
                    qk_tiles[key] = p_qk.tile([128, S], BF16, tag="qk", name="qk_t")
                dst = qk_tiles[key]
                off = half * 1024
                pst = ps.tile([128, 1024], F32, tag="st", name="pp")
                for c in range(CT):
                    wt = wsb[:, c * 256 + pair * 128 : c * 256 + (pair + 1) * 128]
                    for n in range(2):
                        nc.tensor.matmul(
                            pst[:, n * 512 : (n + 1) * 512],
                            lhsT=wt,
                            rhs=xt_t[c][:, off + n * 512 : off + (n + 1) * 512],
                            start=(c == 0),
                            stop=(c == CT - 1),
                        )
                    if warmup and 1 <= c <= 6:
                        # discarded matmuls inside the DMA-wait gaps of the
                        # lead-in: accumulate HAM busy-time so the PE clock
                        # un-throttles before the attention stream starts
                        if not wup_holder:
                            wup_holder.append(
                                ps.tile([128, 512], F32, tag="inj", name="wup")
                            )
                        nc.tensor.matmul(
                            wup_holder[0][:, 0:512],
                            lhsT=wsb[:, 0:128],
                            rhs=xt_t[0][:, 0:512],
                            start=True,
                            stop=True,
                        )
                nc.vector.tensor_copy(dst[:, off : off + 1024], pst[:, 0:1024])

            def proj_qk_quarter(nm, pair, quarter, tag="inj", evict_scalar=False):
                wsb = wq_sb if nm == "q" else wk_sb
                key = (nm, pair)
                if key not in qk_tiles:
                    qk_tiles[key] = p_qk.tile([128, S], BF16, tag="qk", name="qk_t")
                dst = qk_tiles[key]
                off = quarter * 512
                pst = ps.tile([128, 512], F32, tag=tag, name="pq")
                for c in range(CT):
                    wt = wsb[:, c * 256 + pair * 128 : c * 256 + (pair + 1) * 128]
                    nc.tensor.matmul(
                        pst[:, 0:512],
                        lhsT=wt,
                        rhs=xt_t[c][:, off : off + 512],
                        start=(c == 0),
                        stop=(c == CT - 1),
                    )
                if evict_scalar:
                    # loops 1/2 inject quarters at k=9,11 where the DVE is
                    # congested (co-exps + normalize); ScalarE has measured
                    # gap slack exactly there
                    nc.scalar.copy(dst[:, off : off + 512], pst[:, 0:512])
                else:
                    nc.vector.tensor_copy(dst[:, off : off + 512], pst[:, 0:512])

            v_t = []

            def proj_v_tile(st):
                psv = ps.tile([128, 512], F32, tag="inj", name="psv")
                for c in range(CT):
                    nc.tensor.matmul(
                        psv[:, 0 : HL * DQ],
                        lhsT=xt_t[c][:, st * 128 : (st + 1) * 128],
                        rhs=wv_t[c][:],
                        start=(c == 0),
                        stop=(c == CT - 1),
                    )
                vt = p_v.tile([128, HL * 65], BF16, tag="v", name="v_t")
                v3 = vt[:, 0 : HL * 65].rearrange("p (h c) -> p h c", c=65)
                s3 = psv[:, 0 : HL * DQ].rearrange("p (h c) -> p h c", c=DQ)
                o3 = mv_t[:, KT : KT + 4].rearrange("p (h c) -> p h c", c=1)
                # fused eviction+mask: V cols straight from PSUM * mask, and
                # the ones-column (rowsum trick) = 1.0 * mask
                nc.vector.tensor_scalar_mul(
                    v3[:, :, 0:DQ], s3[:, :, :], mv_t[:, st : st + 1]
                )
                nc.vector.tensor_scalar_mul(
                    v3[:, :, DQ : DQ + 1], o3[:, :, :], mv_t[:, st : st + 1]
                )
                v_t.append(vt)

            wup_holder = []
            # Lead-in: ONLY the half-0 projections (pair-0 wq/wk + xt-half0,
            # ~2.5MB of DMA), with Q and K INTERLEAVED per c-chunk so each
            # chunk's 4 matmuls run as soon as that chunk's DMA lands (the
            # in-order tensor queue would otherwise hold all K matmuls
            # behind Q's last chunk).  Discarded warmup matmuls fill the
            # DMA-wait gaps to accumulate HAM busy-time (PE ramps to full
            # clock before the attention stream starts).  xt-half1 lands at
            # ~25us so the half-1 and pair-1 projections are injected inside
            # the attention loops instead.
            def leadin_qk_h0():
                for nm in ("q", "k"):
                    qk_tiles[(nm, 0)] = p_qk.tile([128, S], BF16, tag="qk", name="qk_t")
                pq = ps.tile([128, 1024], F32, tag="st", name="pq0")
                pk = ps.tile([128, 1024], F32, tag="st", name="pk0")
                for c in range(CT):
                    for pst, wsb in ((pq, wq_sb), (pk, wk_sb)):
                        wt = wsb[:, c * 256 : c * 256 + 128]
                        for n in range(2):
                            nc.tensor.matmul(
                                pst[:, n * 512 : (n + 1) * 512],
                                lhsT=wt,
                                rhs=xt_t[c][:, n * 512 : (n + 1) * 512],
                                start=(c == 0),
                                stop=(c == CT - 1),
                            )
                # parallel evictions: q on DVE, k on ScalarE (its DMA
                # issues are done by now; the first exp isn't ready yet)
                nc.vector.tensor_copy(qk_tiles[("q", 0)][:, 0:1024], pq[:, 0:1024])
                nc.scalar.copy(qk_tiles[("k", 0)][:, 0:1024], pk[:, 0:1024])

            leadin_qk_h0()

            # -------- attention: flattened 4-loop software pipeline --------
            # Loop li = pair*2 + j.  Per loop the St/exp stream runs k=0..15
            # on the st tag (rotation paced ONLY by the exp drain).  Head-A
            # PV trails St by PIPE_A on the accA tag; head-B PV starts at
            # SB[li] (its accumulator halves are the LAST inj-tag allocs of
            # the loop, reading pt tiles buffered in SBUF) and its remainder
            # drains 3-per-iteration into the next loop, followed by the
            # accumulator evictions and the normalize chain -- the exp
            # stream never pauses at a j-boundary.  Injected work (V-proj,
            # quarter-projs, out-proj halves, bcast tiles) rides the inj tag
            # between the head-B claims and so never displaces a score tile.
            PIPE_A = 2
            SB = {0: 13, 1: 12, 2: 12, 3: 13}
            # DVE exps sit on head-B tiles (their consumer, head-B PV, runs
            # far later so the 2.44us latency never blocks anything) and are
            # kept AWAY from the k-window where the previous loop's normalize
            # chain occupies the in-order DVE queue (k<=8) -- an exp queued
            # behind 3us of normalize work delays the st-slot rotation.
            # In the PE-thin k=12..15 stretch the loops are ScalarE-paced at
            # ~77%% PE duty, which trips the HAM half-clock gate.  A DVE
            # co-exp every other k there raises the exp drain rate until the
            # PE is the pacer at full duty -- no idle window, no gate.
            DVE_EXP_LI = {
                0: {12, 14},
                1: {9, 11, 12, 14},
                2: {9, 11, 12, 14},
                3: {9, 12, 14},
            }
            # discarded extra score-matmul passes (same psum tile, start=True
            # resets) in the PE-thin k-slots: keeps PE duty >=80% so the HAM
            # clock-gate never drops the PE to half rate mid-pipeline.
            # value = how many heads get a discarded extra pass at that k.
            # NOTE: discarded-matmul filler (doubled-St style) at scale trips
            # a DEVICE power throttle to 2.0GHz (two 301us runs, every op
            # stretched exactly 1.2x) -- real work only.
            # surgical: loop-3 k=5..7 is the one PE-thin stretch (St+pvA
            # only, ~39% duty) that the DVE co-exp trick cannot densify --
            # it stochastically trips a 34us HAM half-clock window over
            # loop 3 + tail (measured 233us draws vs the 217us cluster).
            # 12 discarded matmuls (~1.5% extra PE power) pin it warm; the
            # device power throttle needed ~15%-scale filler to trip.
            ST_REPS = {3: {5: 2, 6: 2, 7: 2}}
            pts = {}
            accA = {}
            accB = {}
            ot_tiles = {}
            evs_store = {}
            rs_tiles = {}
            rc_tiles = {}
            bct_tiles = {}

            def emit_exp(pt, stp, on_dve):
                if on_dve:
                    h = p_h.tile([128, 1024], F32, tag="h", name="h_t")
                    nc.vector._custom_dve(
                        EXP_CUBIC, out=h[:], in0=stp[:],
                        s0=EXPC_S0, s1=EXPC_S1, imm2=EXPC_IMM2,
                    )
                    nc.vector._custom_dve(POW64, out=pt[:], in0=h[:])
                else:
                    nc.scalar.activation(pt[:], stp[:], EXP, scale=1.0 / SCALE)

            def st_exp(li, k):
                pair, j = divmod(li, 2)
                qt = qk_tiles[("q", pair)]
                kt = qk_tiles[("k", pair)]
                jo = j * 1024
                dve_ks = DVE_EXP_LI.get(li, set())
                reps = ST_REPS.get(li, {}).get(k, 0)
                for i, base in enumerate((0, 64)):
                    stp = ps.tile([128, 1024], F32, tag="st", name="stp")
                    for rep in range(1 + (1 if i < reps else 0)):
                        for n in range(2):
                            nc.tensor.matmul(
                                stp[:, n * 512 : (n + 1) * 512],
                                lhsT=kt[base : base + DQ, k * 128 : (k + 1) * 128],
                                rhs=qt[base : base + DQ, jo + n * 512 : jo + (n + 1) * 512],
                                start=True,
                                stop=True,
                            )
                    pt = p_pt.tile(
                        [128, 1024], BF16,
                        tag=("ptA" if i == 0 else "ptB"),
                        bufs=(6 if i == 0 else 16),
                        name="pt_t",
                    )
                    emit_exp(pt, stp, on_dve=(i == 1 and k in dve_ks))
                    pts[(li, k, i)] = pt

            def pv(li, k, i):
                pair, j = divmod(li, 2)
                h = 2 * pair + i
                accs = accA if i == 0 else accB
                if k == 0:
                    accs[li] = [
                        ps.tile([65, 512], F32,
                                tag=("accA" if i == 0 else "inj"), name="acc")
                        for _ in range(2)
                    ]
                pt = pts.pop((li, k, i))
                for n in range(2):
                    nc.tensor.matmul(
                        accs[li][n][0:65, 0:512],
                        lhsT=v_t[k][:, h * 65 : h * 65 + 65],
                        rhs=pt[:, n * 512 : (n + 1) * 512],
                        start=(k == 0),
                        stop=(k == KT - 1),
                    )

            def evict_acc(li, i, hopq=None):
                # acc halves -> ev [65,1024] f32; rowsum row 64 DMA-hops to
                # partition 0 of the loop's [1,2048] rs tile.  At the tail
                # the hops ride the idle gpsimd queue -- the sync queue is
                # backlogged with output DMAs and would delay the recip.
                accs = (accA if i == 0 else accB).pop(li)
                ev = p_ev.tile([65, 1024], F32, tag="ev", name="ev_t")
                if li not in rs_tiles:
                    rs_tiles[li] = p_rs.tile([1, 2048], F32, tag="rs", name="rs_t")
                rs = rs_tiles[li]
                hopq = hopq or nc.sync
                for n in range(2):
                    nc.vector.tensor_copy(
                        ev[0:65, n * 512 : (n + 1) * 512], accs[n][0:65, 0:512]
                    )
                    hopq.dma_start(
                        rs[0:1, i * 1024 + n * 512 : i * 1024 + (n + 1) * 512],
                        ev[64:65, n * 512 : (n + 1) * 512],
                    )
                evs_store[(li, i)] = ev

            def norm_recip(li, i):
                # per-head [1,1024] reciprocal, spread across two k-slots so
                # the in-order DVE queue never holds >1.2us of normalize work
                if li not in rc_tiles:
                    rc_tiles[li] = p_rc.tile([1, 2048], F32, tag="rc", name="rc_t")
                rc = rc_tiles[li]
                nc.vector.reciprocal_approx_fast(
                    rc[0:1, i * 1024 : (i + 1) * 1024],
                    rs_tiles[li][0:1, i * 1024 : (i + 1) * 1024],
                )
                if i == 1:
                    rs_tiles.pop(li)

            def norm_bcast(li):
                # mid-loop broadcast of 1/rowsum on the otherwise-idle
                # gpsimd: the PE-matmul broadcast would sit behind ~12
                # k-iterations of queued matmuls, and the DVE multiplies
                # waiting on it fill the DVE 4-deep wait queue and block the
                # k=12/14 co-exps (measured 8.5us exp gap).
                bct = p_bc.tile([64, 2048], F32, tag="bc", name="bc_t")
                nc.gpsimd.partition_broadcast(bct[0:64, 0:2048], rc_tiles[li][0:1, 0:2048])
                bct_tiles[li] = bct

            def norm_apply_bct(li, i):
                if li not in ot_tiles:
                    ot_tiles[li] = p_ot.tile([128, 1024], BF16, tag="ot", name="ot_t")
                ot = ot_tiles[li]
                bct = bct_tiles[li]
                ev = evs_store.pop((li, i))
                nc.vector.tensor_mul(
                    ot[i * 64 : (i + 1) * 64, 0:1024],
                    ev[0:64, 0:1024],
                    bct[0:64, i * 1024 : (i + 1) * 1024],
                )
                if i == 1:
                    rc_tiles.pop(li)
                    bct_tiles.pop(li)

            def norm_apply_head(li, i):
                # O = PV/rowsum via PE partition-broadcast of 1/rowsum (tiny
                # ones[1,64]^T @ rc matmuls into inj-tag psum) + DVE multiply;
                # head A -> ot rows 0-63, head B -> rows 64-127.  (gpsimd
                # cannot read PSUM, so the multiply must stay on DVE.)
                if li not in ot_tiles:
                    ot_tiles[li] = p_ot.tile([128, 1024], BF16, tag="ot", name="ot_t")
                ot = ot_tiles[li]
                rc = rc_tiles[li]
                ev = evs_store.pop((li, i))
                for n in range(2):
                    bcp = ps.tile([64, 512], F32, tag="inj", name="bcp")
                    nc.tensor.matmul(
                        bcp[0:64, 0:512],
                        lhsT=ones64[0:1, 0:64],
                        rhs=rc[0:1, i * 1024 + n * 512 : i * 1024 + (n + 1) * 512],
                        start=True,
                        stop=True,
                    )
                    nc.vector.tensor_mul(
                        ot[i * 64 : (i + 1) * 64, n * 512 : (n + 1) * 512],
                        ev[0:64, n * 512 : (n + 1) * 512],
                        bcp[0:64, 0:512],
                    )
                if i == 1:
                    rc_tiles.pop(li)

            def norm_apply(li, tail=False):
                norm_apply_head(li, 0)
                norm_apply_head(li, 1)

            def outproj_half(st, n, tail=False):
                po = ps.tile([128, 512], F32, tag="inj", name="po")
                sj, so = st // 8, (st % 8) * 128
                for p in range(2):
                    nc.tensor.matmul(
                        po[:, 0:512],
                        lhsT=ot_tiles[p * 2 + sj][:, so : so + 128],
                        rhs=wo_t[p][:, n * 512 : (n + 1) * 512],
                        start=(p == 0),
                        stop=(p == 1),
                    )
                os_t = p_os.tile([128, 512], BF16, tag="os", name="os_t")
                if tail and n == 1:
                    nc.scalar.copy(os_t[:, 0:512], po[:, 0:512])
                else:
                    nc.vector.tensor_copy(os_t[:, 0:512], po[:, 0:512])
                # mid-loop output DMAs stay off the scalar queue (a
                # scalar.dma_start would block the exp stream); the tail
                # stripes across both HWDGE paths
                qd = nc.scalar if (tail and (st + n) % 2 == 1) else nc.sync
                qd.dma_start(
                    OUT[st * 128 : (st + 1) * 128, n * 512 : (n + 1) * 512],
                    os_t[:, 0:512],
                )

            # per-loop event schedule: events[li][k] = thunks run AFTER
            # st_exp(li, k).  Build order matters within a k-slot: PV trails
            # and drains first, then acc evictions, then injections (an inj
            # alloc emitted before the eviction that frees its slot would
            # head-of-line-block the tensor queue).
            events = [defaultdict(list) for _ in range(4)]

            def at(li, k, th):
                events[li][k].append(th)

            mk = lambda f, *a, **kw: (lambda: f(*a, **kw))

            for li in range(4):
                for k in range(KT):
                    if k - PIPE_A >= 0:
                        at(li, k, mk(pv, li, k - PIPE_A, 0))
                    if k - SB[li] >= 0:
                        at(li, k, mk(pv, li, k - SB[li], 1))
                if li < 3:
                    at(li + 1, 0, mk(pv, li, 14, 0))
                    at(li + 1, 1, mk(pv, li, 15, 0))
                    rem = list(range(KT - SB[li], KT))
                    dk = 0
                    while rem:
                        for b in rem[:3]:
                            at(li + 1, dk, mk(pv, li, b, 1))
                        rem = rem[3:]
                        dk += 1
                    at(li + 1, 2, mk(evict_acc, li, 0))
                    at(li + 1, 3, mk(norm_recip, li, 0))
                    at(li + 1, dk, mk(evict_acc, li, 1))
                    at(li + 1, dk + 1, mk(norm_recip, li, 1))
                    at(li + 1, dk + 2, mk(norm_bcast, li))
                    at(li + 1, dk + 3, mk(norm_apply_bct, li, 0))
                    at(li + 1, dk + 4, mk(norm_apply_bct, li, 1))

            # loop-0 injections: V-proj (16 tiles; wv lands ~20us so st0
            # waits for k=1; st8+ read xt-half1, at k>=7), K0-half1 quarters
            # (deadlines k=8/k=12), Q0-half1 via st-tag displacement late in
            # the loop (it is PE-bound there; the exp stream has slack)
            for kk, sts in {0: (0, 2), 1: (2, 4), 2: (4, 6), 3: (6, 8),
                            4: (8, 10), 7: (10, 12), 8: (12, 14),
                            9: (14, 16)}.items():
                for st in range(*sts):
                    at(0, kk, mk(proj_v_tile, st))
            at(0, 5, mk(proj_qk_quarter, "k", 0, 2))
            at(0, 6, mk(proj_qk_quarter, "k", 0, 3))
            at(0, 10, mk(proj_qk_quarter, "q", 0, 2))
            at(0, 11, mk(proj_qk_quarter, "q", 0, 3))
            # loop-1 injections: pair-1 Q/K half-0 quarters (needed by
            # loop 2), spread k=5..11 to fill the PE-thin post-drain region
            at(1, 5, mk(proj_qk_quarter, "q", 1, 0))
            at(1, 7, mk(proj_qk_quarter, "q", 1, 1))
            at(1, 9, mk(proj_qk_quarter, "k", 1, 0, evict_scalar=True))
            at(1, 11, mk(proj_qk_quarter, "k", 1, 1, evict_scalar=True))
            # loop-2 injections: pair-1 half-1 quarters (K needed by its own
            # k=8/k=12, Q by loop 3)
            at(2, 5, mk(proj_qk_quarter, "k", 1, 2))
            at(2, 7, mk(proj_qk_quarter, "k", 1, 3))
            at(2, 9, mk(proj_qk_quarter, "q", 1, 2, evict_scalar=True))
            at(2, 11, mk(proj_qk_quarter, "q", 1, 3, evict_scalar=True))
            # loop-3 injections: out-proj s-slices 0-4 (need ot10, ready ~k8)
            for idx, st in enumerate(range(0, 5)):
                at(3, 8 + idx, mk(outproj_half, st, 0))
                at(3, 8 + idx, mk(outproj_half, st, 1))

            for li in range(4):
                for k in range(KT):
                    st_exp(li, k)
                    for th in events[li][k]:
                        th()

            # ---------------- tail: loop-3 drain + out-proj 5-15 ------------
            # Scores are done, so the 4-bank st tag is repurposed for FULL
            # [128,1024] out-proj slices (2-slot ping-pong, evictions split
            # DVE/ScalarE in parallel) -- the 1-bank inj rotation would pace
            # at ~1.2us/half-slice.  Slices 5-7 depend only on ot00/ot10, so
            # their matmuls interleave with the head-B drain and keep the PE
            # warm through the normalize chain.
            def outproj_slice_tail(st):
                # even slices: one [128,1024] st-tag psum; odd slices: two
                # [128,512] accA-tag halves -> a 4-deep slot pipeline
                sj, so = st // 8, (st % 8) * 128
                os_t = p_os.tile([128, 1024], BF16, tag="osf", bufs=4, name="os_t")
                if st % 2 == 0:
                    full = ps.tile([128, 1024], F32, tag="st", name="po")
                    dsts = [full[:, 0:512], full[:, 512:1024]]
                else:
                    dsts = [
                        ps.tile([128, 512], F32, tag="accA", name="poh")[:, 0:512]
                        for _ in range(2)
                    ]
                for n in range(2):
                    for p in range(2):
                        nc.tensor.matmul(
                            dsts[n],
                            lhsT=ot_tiles[p * 2 + sj][:, so : so + 128],
                            rhs=wo_t[p][:, n * 512 : (n + 1) * 512],
                            start=(p == 0),
                            stop=(p == 1),
                        )
                    if n == 0:
                        nc.vector.tensor_copy(os_t[:, 0:512], dsts[0])
                    else:
                        nc.scalar.copy(os_t[:, 512:1024], dsts[1])
                    qd = nc.sync if n == 0 else nc.scalar
                    qd.dma_start(
                        OUT[st * 128 : (st + 1) * 128, n * 512 : (n + 1) * 512],
                        os_t[:, n * 512 : (n + 1) * 512],
                    )

            pv(3, 14, 0)
            pv(3, 15, 0)
            evict_acc(3, 0)
            norm_recip(3, 0)
            for b in range(KT - SB[3], KT):
                pv(3, b, 1)
            evict_acc(3, 1)
            norm_recip(3, 1)
            norm_apply_head(3, 0)
            outproj_slice_tail(5)
            outproj_slice_tail(6)
            # 3 discarded matmuls bridge the recipB->bcpB latency: the ~2us
            # PE hole here is what trips the tail half-clock window
            brid = ps.tile([128, 512], F32, tag="st", name="brid")
            for _ in range(3):
                nc.tensor.matmul(
                    brid[:, 0:512], lhsT=wq_sb[:, 0:128], rhs=wq_sb[:, 0:512],
                    start=True, stop=True,
                )
            norm_apply_head(3, 1)
            outproj_slice_tail(7)
            for st in range(8, KT):
                outproj_slice_tail(st)
